# Initial kernel scaffold
#
"""LIF (leaky integrate-and-fire) forward scan on 8 Trainium2 NeuronCores.

Reference recurrence (per element, scan over T):
    m_t = v_{t-1} * tau + x_t
    y_t = (m_t - v_th > 0) ? 1.0 : 0.0
    v_t = m_t * (1 - y_t)          # hard reset on spike

x: [T=16, B=32, C=128, H=32, W=32] f32.  Data-parallel over B: each core
gets B_loc=4 batches -> per-step tile [C=128 partitions, B_loc*H*W=4096].

Per step, 3 DVE ops via scalar_tensor_tensor / tensor_scalar:
    m  = (v mult tau) add x_t        (STT)
    y  = (m is_gt v_th)              (TS, 2x mode)
    v' = (m is_le v_th) mult m       (STT)
"""

import sys

sys.path.insert(0, "/opt/trn_rl_repo")

from contextlib import ExitStack

import numpy as np

import concourse.bass as bass
import concourse.tile as tile
from concourse import bacc, mybir
from concourse.bass_utils import run_bass_kernel_spmd

# Hyperparameters (from the nn.Module)
V_TH = 1.0
TAU = 0.5

# Shapes (hardcoded per problem spec)
T, B, C, H, W = 16, 32, 128, 32, 32
N_CORES = 8
B_LOC = B // N_CORES           # 4 batches per core
S = H * W                      # 1024 spatial sites
FREE = B_LOC * S               # 4096 free-dim elements per step tile

DT = mybir.dt.float32

# Column chunking of the per-step tile (1 = single [128, 4096] tile)
N_CHUNKS = 2
CHUNK = FREE // N_CHUNKS       # free-dim columns per chunk
B_PER_CHUNK = B_LOC // N_CHUNKS


def build_kernel(repeats: int = 1) -> bass.Bass:
    """Build the LIF program. repeats>1 chains extra body copies (carrying v
    across the repeat boundary) for steady-state HW benchmarking."""
    nc = bacc.Bacc(
        "TRN2", target_bir_lowering=False, debug=False, num_devices=N_CORES
    )
    x_d = nc.dram_tensor("x", [T, B_LOC, C, S], DT, kind="ExternalInput").ap()
    # y is binary {0,1}: store as uint8 (4x less HBM write traffic); host
    # converts back to f32 exactly.
    y_d = nc.dram_tensor(
        "y", [T, B_LOC, C, S], mybir.dt.uint8, kind="ExternalOutput"
    ).ap()

    # Register a -V_TH const AP (activation bias needs a [128,1] SBUF const).
    _c = nc.alloc_sbuf_tensor(f"const-float32-{-V_TH}", [128, 1], DT)
    nc.gpsimd.memset(_c.ap(), -V_TH)
    nc.const_aps.aps[(DT, -V_TH)] = _c.ap()
    nc.all_engine_barrier()

    with ExitStack() as ctx:
        tc = ctx.enter_context(tile.TileContext(nc))
        x_pool = ctx.enter_context(tc.tile_pool(name="x", bufs=5))
        y_pool = ctx.enter_context(tc.tile_pool(name="y", bufs=4))
        m_pool = ctx.enter_context(tc.tile_pool(name="m", bufs=3))
        v_pool = ctx.enter_context(tc.tile_pool(name="v", bufs=2))
        s_pool = ctx.enter_context(tc.tile_pool(name="s", bufs=2))

        # One independent LIF chain per column chunk.
        v_cur = [None] * N_CHUNKS
        for r in range(repeats):
            for t in range(T):
                for k in range(N_CHUNKS):
                    b0 = k * B_PER_CHUNK
                    # DRAM slice [B_PER_CHUNK, C, S] -> partition-major view
                    x_src = x_d[t, b0 : b0 + B_PER_CHUNK].rearrange("b c s -> c b s")
                    y_dst = y_d[t, b0 : b0 + B_PER_CHUNK].rearrange("b c s -> c b s")

                    xt = x_pool.tile([C, B_PER_CHUNK, S], DT, tag="x")
                    nc.sync.dma_start(out=xt[:], in_=x_src)
                    xt2 = xt[:].rearrange("c b s -> c (b s)")

                    mt = m_pool.tile([C, CHUNK], DT, tag="m")
                    if t == 0 and r == 0:
                        # v0 = 0 -> m_0 = x_0 (tensor_scalar add 0 = copy)
                        nc.vector.tensor_scalar(
                            mt[:], xt2, 0.0, None, mybir.AluOpType.add
                        )
                    else:
                        nc.vector.scalar_tensor_tensor(
                            mt[:], v_cur[k][:], TAU, xt2,
                            mybir.AluOpType.mult, mybir.AluOpType.add,
                        )
                    m = mt[:]

                    # y = (m > v_th) exactly, off the DVE critical path:
                    # ACT: s = Sign(m - v_th) in {-1,0,1}; y = Relu(s) in {0,1}
                    st = s_pool.tile([C, CHUNK], DT, tag="s")
                    nc.scalar.activation(
                        st[:], m, mybir.ActivationFunctionType.Sign, bias=-V_TH
                    )
                    yt = y_pool.tile([C, CHUNK], mybir.dt.uint8, tag="y")
                    nc.scalar.activation(
                        yt[:], st[:], mybir.ActivationFunctionType.Relu
                    )
                    nc.scalar.dma_start(
                        out=y_dst,
                        in_=yt[:].rearrange("c (b s) -> c b s", b=B_PER_CHUNK),
                    )

                    if t < T - 1 or r < repeats - 1:
                        vt = v_pool.tile([C, CHUNK], DT, tag="v")
                        nc.vector.scalar_tensor_tensor(
                            vt[:], m, V_TH, m,
                            mybir.AluOpType.is_le, mybir.AluOpType.mult,
                        )
                        v_cur[k] = vt
    nc.finalize()
    return nc


_NC_CACHE = None


def _get_nc():
    global _NC_CACHE
    if _NC_CACHE is None:
        _NC_CACHE = build_kernel()
    return _NC_CACHE


def kernel(x: np.ndarray) -> np.ndarray:
    assert x.shape == (T, B, C, H, W), x.shape
    in_dtype = x.dtype
    xf = np.asarray(x, dtype=np.float32).reshape(T, B, C, S)

    nc = _get_nc()
    in_maps = [
        {"x": np.ascontiguousarray(xf[:, k * B_LOC : (k + 1) * B_LOC])}
        for k in range(N_CORES)
    ]
    res = run_bass_kernel_spmd(nc, in_maps, list(range(N_CORES)))
    out = np.concatenate([res.results[k]["y"] for k in range(N_CORES)], axis=1)
    return out.reshape(T, B, C, H, W).astype(in_dtype, copy=False)


if __name__ == "__main__":
    x = np.random.randn(T, B, C, H, W).astype(np.float32)
    y = kernel(x)
    print("out", y.shape, y.dtype, "spike rate", y.mean())



# revision 1
# speedup vs baseline: 1.2544x; 1.2544x over previous
"""LIF (leaky integrate-and-fire) forward scan on 8 Trainium2 NeuronCores.

Reference recurrence (per element, scan over T):
    m_t = v_{t-1} * tau + x_t
    y_t = (m_t - v_th > 0) ? 1.0 : 0.0
    v_t = m_t * (1 - y_t)          # hard reset on spike

x: [T=16, B=32, C=128, H=32, W=32] f32.  Data-parallel over B: each core
gets B_loc=4 batches -> per-step tile [C=128 partitions, B_loc*H*W=4096].

Per step, 3 DVE ops via scalar_tensor_tensor / tensor_scalar:
    m  = (v mult tau) add x_t        (STT)
    y  = (m is_gt v_th)              (TS, 2x mode)
    v' = (m is_le v_th) mult m       (STT)
"""

import sys

sys.path.insert(0, "/opt/trn_rl_repo")

from contextlib import ExitStack

import numpy as np

import concourse.bass as bass
import concourse.tile as tile
from concourse import bacc, mybir
from concourse.bass_utils import run_bass_kernel_spmd

# Hyperparameters (from the nn.Module)
V_TH = 1.0
TAU = 0.5

# Shapes (hardcoded per problem spec)
T, B, C, H, W = 16, 32, 128, 32, 32
N_CORES = 8
B_LOC = B // N_CORES           # 4 batches per core
S = H * W                      # 1024 spatial sites
FREE = B_LOC * S               # 4096 free-dim elements per step tile

DT = mybir.dt.float32

# Column chunking of the per-step tile (1 = single [128, 4096] tile)
N_CHUNKS = 2
CHUNK = FREE // N_CHUNKS       # free-dim columns per chunk
B_PER_CHUNK = B_LOC // N_CHUNKS


def build_kernel(repeats: int = 1) -> bass.Bass:
    """Build the LIF program. repeats>1 chains extra body copies (carrying v
    across the repeat boundary) for steady-state HW benchmarking."""
    nc = bacc.Bacc(
        "TRN2", target_bir_lowering=False, debug=False, num_devices=N_CORES
    )
    x_d = nc.dram_tensor("x", [T, B_LOC, C, S], DT, kind="ExternalInput").ap()
    # y is binary {0,1}: store as uint8 (4x less HBM write traffic); host
    # converts back to f32 exactly.
    y_d = nc.dram_tensor(
        "y", [T, B_LOC, C, S], mybir.dt.uint8, kind="ExternalOutput"
    ).ap()

    # Register a -V_TH const AP (activation bias needs a [128,1] SBUF const).
    _c = nc.alloc_sbuf_tensor(f"const-float32-{-V_TH}", [128, 1], DT)
    nc.gpsimd.memset(_c.ap(), -V_TH)
    nc.const_aps.aps[(DT, -V_TH)] = _c.ap()
    nc.all_engine_barrier()

    with ExitStack() as ctx:
        tc = ctx.enter_context(tile.TileContext(nc))
        x_pool = ctx.enter_context(tc.tile_pool(name="x", bufs=5))
        y_pool = ctx.enter_context(tc.tile_pool(name="y", bufs=4))
        m_pool = ctx.enter_context(tc.tile_pool(name="m", bufs=3))
        v_pool = ctx.enter_context(tc.tile_pool(name="v", bufs=2))
        s_pool = ctx.enter_context(tc.tile_pool(name="s", bufs=2))

        # One independent LIF chain per column chunk.
        v_cur = [None] * N_CHUNKS
        for r in range(repeats):
            for t in range(T):
                for k in range(N_CHUNKS):
                    b0 = k * B_PER_CHUNK
                    # DRAM slice [B_PER_CHUNK, C, S] -> partition-major view
                    x_src = x_d[t, b0 : b0 + B_PER_CHUNK].rearrange("b c s -> c b s")
                    y_dst = y_d[t, b0 : b0 + B_PER_CHUNK].rearrange("b c s -> c b s")

                    xt = x_pool.tile([C, B_PER_CHUNK, S], DT, tag="x")
                    nc.sync.dma_start(out=xt[:], in_=x_src)
                    xt2 = xt[:].rearrange("c b s -> c (b s)")

                    mt = m_pool.tile([C, CHUNK], DT, tag="m")
                    if t == 0 and r == 0:
                        # v0 = 0 -> m_0 = x_0 (tensor_scalar add 0 = copy)
                        nc.vector.tensor_scalar(
                            mt[:], xt2, 0.0, None, mybir.AluOpType.add
                        )
                    else:
                        nc.vector.scalar_tensor_tensor(
                            mt[:], v_cur[k][:], TAU, xt2,
                            mybir.AluOpType.mult, mybir.AluOpType.add,
                        )
                    m = mt[:]

                    # y = (m > v_th) exactly, off the DVE critical path:
                    # ACT: s = Sign(m - v_th) in {-1,0,1}; y = Relu(s) in {0,1}
                    st = s_pool.tile([C, CHUNK], DT, tag="s")
                    nc.scalar.activation(
                        st[:], m, mybir.ActivationFunctionType.Sign, bias=-V_TH
                    )
                    yt = y_pool.tile([C, CHUNK], mybir.dt.uint8, tag="y")
                    nc.scalar.activation(
                        yt[:], st[:], mybir.ActivationFunctionType.Relu
                    )
                    nc.scalar.dma_start(
                        out=y_dst,
                        in_=yt[:].rearrange("c (b s) -> c b s", b=B_PER_CHUNK),
                    )

                    if t < T - 1 or r < repeats - 1:
                        vt = v_pool.tile([C, CHUNK], DT, tag="v")
                        nc.vector.scalar_tensor_tensor(
                            vt[:], m, V_TH, m,
                            mybir.AluOpType.is_le, mybir.AluOpType.mult,
                        )
                        v_cur[k] = vt
    nc.finalize()
    return nc


_NC_CACHE = None


def _get_nc():
    global _NC_CACHE
    if _NC_CACHE is None:
        _NC_CACHE = build_kernel()
    return _NC_CACHE


def kernel(x: np.ndarray) -> np.ndarray:
    assert x.shape == (T, B, C, H, W), x.shape
    in_dtype = x.dtype
    xf = np.asarray(x, dtype=np.float32).reshape(T, B, C, S)

    nc = _get_nc()
    in_maps = [
        {"x": np.ascontiguousarray(xf[:, k * B_LOC : (k + 1) * B_LOC])}
        for k in range(N_CORES)
    ]
    res = run_bass_kernel_spmd(nc, in_maps, list(range(N_CORES)))
    out = np.concatenate([res.results[k]["y"] for k in range(N_CORES)], axis=1)
    return out.reshape(T, B, C, H, W).astype(in_dtype, copy=False)


if __name__ == "__main__":
    x = np.random.randn(T, B, C, H, W).astype(np.float32)
    y = kernel(x)
    print("out", y.shape, y.dtype, "spike rate", y.mean())

